# revision 1
# baseline (speedup 1.0000x reference)
"""Causal self-attention with reference-feature cross keys, on 8 TRN2 cores.

Sharding: tensor-parallel over heads. Core c owns global heads (2c, 2c+1),
i.e. columns [128c:128c+128) of Wq/Wk/Wv/Wrk/Wrv and rows [128c:128c+128)
of Wp. Each core returns a partial y; the host sums the 8 partials and adds
bp (the "all-reduce").

The host feeds x and ref_feat already transposed ([C, tokens]) so the
contraction dim lands on SBUF partitions with plain DMAs — no on-device
transposes of x. All big matmuls run in float32r (~1.3e-4 rel err, 1 cyc/row
at N>=512 vs 4 for fp32); f32r tiles are DMA-loaded directly (walrus
accepts dtype-matched DMA producers).

Per-core dataflow:
  qT/kT/vT [128, tok] = W.T @ xT          (accumulate over 8 C-chunks)
  v natural via PE transpose of vT blocks (+ ones column for the denom)
  S^T[s, t] = kT_blk.T @ qT_chunk         scores transposed; both heads in
                                          one 2-bank PSUM group (K=64 row
                                          packing via base_partition 0/64)
  E = exp(S^T / 8)                        one ACT instr per block pair; no
                                          max-subtraction needed (|S| < ~3)
  diag blocks masked multiplicatively on GPSIMD (host mask tiles)
  O^T (+denom row) += [V | 1].T @ E       accumulated over s-blocks
  O^T /= denom  (DVE reciprocal + GPSIMD partition_broadcast + DVE mult)
  y_part[t, :] = O^T_blk.T @ Wp_rows      (O^T is directly the stationary)
"""
import sys

sys.path.insert(0, "/opt/trn_rl_repo")

import numpy as np

B, T, C = 4, 2048, 1024
TR = 512
D = 64
DC = 128          # per-core slice of C (2 heads x 64)
H_PER = 2
NCH = T // 512    # 512-token chunks per batch
NCORES = 8

_CACHE = {}


def _build_program(repeat=1, ablate="none"):
    import concourse.bacc as bacc
    import concourse.mybir as mybir
    import concourse.tile as tile
    from concourse.masks import make_identity

    F32 = mybir.dt.float32
    F32R = mybir.dt.float32r
    AF = mybir.ActivationFunctionType
    OP = mybir.AluOpType

    nc = bacc.Bacc("TRN2", target_bir_lowering=False, debug=False,
                   num_devices=NCORES)

    xt_d = nc.dram_tensor("xt", [C, B * T], F32R, kind="ExternalInput").ap()
    rt_d = nc.dram_tensor("rt", [C, B * TR], F32R, kind="ExternalInput").ap()
    w_d = {}
    for nm in ("wq", "wk", "wv", "wrk", "wrv"):
        w_d[nm] = nc.dram_tensor(nm, [C, DC], F32R, kind="ExternalInput").ap()
    wp_d = nc.dram_tensor("wp", [DC, C], F32R, kind="ExternalInput").ap()
    b_d = {}
    for nm in ("bq", "bk", "bv", "brk", "brv"):
        b_d[nm] = nc.dram_tensor(nm, [DC], F32, kind="ExternalInput").ap()
    msk_d = nc.dram_tensor("masks", [128, 4, 512], F32, kind="ExternalInput").ap()
    out_d = nc.dram_tensor("out", [B, T, C], F32, kind="ExternalOutput").ap()

    xt_v = xt_d.rearrange("(co p) t -> p co t", p=128)
    rt_v = rt_d.rearrange("(co p) t -> p co t", p=128)

    with tile.TileContext(nc) as tc:
        with (
            tc.tile_pool(name="const", bufs=1) as constp,
            tc.tile_pool(name="work", bufs=2) as work,
            tc.tile_pool(name="psum", bufs=1, space="PSUM") as psp,
        ):
            ident = constp.tile([128, 128], F32)
            make_identity(nc, ident[:])
            ones_col = constp.tile([128, 16], F32)
            nc.any.memset(ones_col[:], 1.0)
            msk = constp.tile([128, 4, 512], F32)
            nc.sync.dma_start(msk[:], msk_d)

            w_sb = {}
            for nm in ("wq", "wk", "wv", "wrk", "wrv"):
                w = constp.tile([128, 8, DC], F32R, name=f"{nm}_sb")
                nc.sync.dma_start(w[:], w_d[nm].rearrange("(co p) m -> p co m", p=128))
                w_sb[nm] = w
            wp_r = constp.tile([DC, C], F32R)
            nc.sync.dma_start(wp_r[:], wp_d)

            b_sb = {}
            for nm in ("bq", "bk", "bv", "brk", "brv"):
                bias = constp.tile([DC, 1], F32, name=f"{nm}_sb")
                nc.sync.dma_start(bias[:], b_d[nm].unsqueeze(1))
                b_sb[nm] = bias

            import contextlib
            rep_ctx = tc.For_i(0, repeat, 1) if repeat > 1 else contextlib.nullcontext()
            with rep_ctx:
              for b in range(B):
                  qT = work.tile([128, NCH, 512], F32R, tag="qT")
                  kT = work.tile([128, NCH, 512], F32R, tag="kT")
                  v_sb = work.tile([128, 4 * NCH, 132], F32R, tag="vsb")
                  nc.vector.tensor_copy(v_sb[:, :, 64:65], ones_col[:, :, None])
                  nc.vector.tensor_copy(v_sb[:, :, 130:131], ones_col[:, :, None])
                  rkT = work.tile([128, 512], F32R, tag="rkT")
                  rv_sb = work.tile([128, 4, 132], F32R, tag="rvsb")
                  nc.vector.tensor_copy(rv_sb[:, :, 64:65], ones_col[:, 0:4, None])
                  nc.vector.tensor_copy(rv_sb[:, :, 130:131], ones_col[:, 0:4, None])
                  OT = work.tile([128, NCH, 512], F32R, tag="OT")

                  def project(xT, wname, bname, dst):
                      """dst[128, 512] (f32r) = W.T @ xT + bias."""
                      if ablate == "noproj":
                          nc.vector.tensor_copy(dst, xT[:, 0, :])
                          return
                      pp = psp.tile([128, 1024], F32, tag="s", bufs=3)
                      for co in range(8):
                          nc.tensor.matmul(pp[:, 0:512], w_sb[wname][:, co, :],
                                           xT[:, co, :], start=(co == 0), stop=(co == 7))
                      nc.vector.tensor_scalar_add(dst, pp[:, 0:512], b_sb[bname][:])

                  def v_natural(vT, dst_vsb, j0):
                      """Transpose vT [128, 512] into v_sb blocks j0..j0+3 (+ones cols)."""
                      pt = psp.tile([128, 1024], F32, tag="s", bufs=3)
                      for a in range(4):
                          nc.tensor.transpose(
                              pt[:, 128 * a:128 * (a + 1)],
                              vT[:, 128 * a:128 * (a + 1)].bitcast(F32), ident[:])
                      ptv = pt[:, 0:512].rearrange("p (a m) -> p a m", a=4)
                      nc.vector.tensor_copy(dst_vsb[:, j0:j0 + 4, 0:64], ptv[:, :, 0:64])
                      nc.vector.tensor_copy(dst_vsb[:, j0:j0 + 4, 66:130], ptv[:, :, 64:128])

                  # ---- projections over x[b] ----
                  for n in range(NCH):
                      t0 = b * T + 512 * n
                      xT = work.tile([128, 8, 512], F32R, tag="xT", bufs=3)
                      nc.sync.dma_start(xT[:], xt_v[:, :, t0:t0 + 512])
                      project(xT, "wq", "bq", qT[:, n, :])
                      project(xT, "wk", "bk", kT[:, n, :])
                      vT = work.tile([128, 512], F32R, tag="vT")
                      project(xT, "wv", "bv", vT[:])
                      v_natural(vT, v_sb, 4 * n)
                  # ---- ref projections ----
                  refT = work.tile([128, 8, 512], F32R, tag="xT", bufs=3)
                  nc.sync.dma_start(refT[:], rt_v[:, :, b * TR:(b + 1) * TR])
                  project(refT, "wrk", "brk", rkT[:])
                  rvT = work.tile([128, 512], F32R, tag="vT")
                  project(refT, "wrv", "brv", rvT[:])
                  v_natural(rvT, rv_sb, 0)

                  # ---- attention: both heads per block (K=64 row packing) ----
                  if ablate == "noattn":
                      for c in range(NCH):
                          nc.vector.tensor_copy(OT[:, c, :], qT[:, c, :])
                      attn_chunks = []
                  else:
                      attn_chunks = list(range(NCH))
                  DEPTH = 2
                  for c in attn_chunks:
                      po = [psp.tile([128, 512], F32, tag="po", bufs=2, name=f"po{h}")
                            for h in range(H_PER)]
                      # (kind, idx, mask_r): self blocks then ref blocks
                      blocks = [("self", j, j - 4 * c if j >= 4 * c else None)
                                for j in range(4 * c + 4)]
                      blocks += [("ref", jr, None) for jr in range(4)]
                      nb = len(blocks)
                      Es = {}

                      def s_stage(bi):
                          kind, j, r = blocks[bi]
                          ps = psp.tile([128, 2, 512], F32, tag="s", bufs=3)
                          for h in (() if ablate == "nos" else range(H_PER)):
                              if kind == "self":
                                  stat = kT[64 * h:64 * (h + 1), j // 4,
                                            128 * (j % 4):128 * (j % 4 + 1)]
                              else:
                                  stat = rkT[64 * h:64 * (h + 1), 128 * j:128 * (j + 1)]
                              nc.tensor.matmul(ps[:, h, :], stat,
                                               qT[64 * h:64 * (h + 1), c, :],
                                               start=True, stop=True)
                          E = work.tile([128, 2, 512], F32R, tag="E",
                                        bufs=DEPTH + 4)
                          if ablate == "noexp":
                              nc.vector.tensor_copy(E[:], ps[:])
                          else:
                              nc.scalar.activation(E[:], ps[:], AF.Exp, scale=0.125)
                          if r is not None:
                              nc.gpsimd.tensor_tensor(
                                  E[:], E[:],
                                  msk[:, r:r + 1, :].to_broadcast((128, 2, 512)),
                                  OP.mult)
                          Es[bi] = E

                      def pv_stage(bi):
                          kind, j, r = blocks[bi]
                          E = Es.pop(bi)
                          if ablate == "nopv":
                              if bi == 0:
                                  for h in range(H_PER):
                                      nc.tensor.matmul(po[h][0:65, :],
                                                       v_sb[:, 0, 66 * h:66 * h + 65],
                                                       E[:, h, :], start=True, stop=True)
                              return
                          for h in range(H_PER):
                              vstat = (v_sb[:, j, 66 * h:66 * h + 65] if kind == "self"
                                       else rv_sb[:, j, 66 * h:66 * h + 65])
                              nc.tensor.matmul(po[h][0:65, :], vstat, E[:, h, :],
                                               start=(bi == 0), stop=(bi == nb - 1))

                      for bi in range(min(DEPTH, nb)):
                          s_stage(bi)
                      for bi in range(nb):
                          if bi + DEPTH < nb:
                              s_stage(bi + DEPTH)
                          pv_stage(bi)
                      for h in range(H_PER):
                          rec = work.tile([1, 512], F32R, tag="rec", bufs=2)
                          with nc.allow_low_precision(reason="f32r ~19 mantissa bits"):
                              nc.vector.reciprocal(rec[:], po[h][64:65, :])
                          bc_sb = work.tile([64, 512], F32R, tag="bc", bufs=2)
                          nc.gpsimd.partition_broadcast(bc_sb[:], rec[:])
                          nc.vector.tensor_tensor(OT[64 * h:64 * (h + 1), c, :],
                                                  po[h][0:64, :], bc_sb[:], OP.mult)

                  # ---- output projection: y_part = O^T_blk.T @ Wp_rows ----
                  for c in range(NCH):
                      for a in range(4):
                          stat = OT[:, c, 128 * a:128 * (a + 1)]
                          y_sb = work.tile([128, C], F32, tag="y")
                          py = psp.tile([128, 1024], F32, tag="s", bufs=3)
                          for half in range(2):
                              nc.tensor.matmul(py[:, 512 * half:512 * (half + 1)], stat,
                                               wp_r[:, 512 * half:512 * (half + 1)],
                                               start=True, stop=True)
                          nc.vector.tensor_copy(y_sb[:], py[:])
                          t0 = 512 * c + 128 * a
                          nc.sync.dma_start(out_d[b, t0:t0 + 128, :], y_sb[:])

    nc.compile()
    return nc


def _get_program(repeat=1, ablate="none"):
    key = ("nc", repeat, ablate)
    if key not in _CACHE:
        _CACHE[key] = _build_program(repeat, ablate)
    return _CACHE[key]


def _make_masks():
    s = np.arange(128)[:, None]
    t = np.arange(512)[None, :]
    return np.stack([(t >= s + 128 * r) for r in range(4)], axis=1).astype(np.float32)


def make_in_maps(x, ref_feat, Wq, bq, Wk, bk, Wv, bv, Wrk, brk, Wrv, brv, Wp, bp):
    x = np.asarray(x, dtype=np.float32)
    ref_feat = np.asarray(ref_feat, dtype=np.float32)
    xt = np.ascontiguousarray(x.reshape(B * T, C).T)
    rt = np.ascontiguousarray(ref_feat.reshape(B * TR, C).T)
    masks = _make_masks()

    in_maps = []
    for c in range(NCORES):
        sl = slice(DC * c, DC * (c + 1))
        in_maps.append({
            "xt": xt, "rt": rt, "masks": masks,
            "wq": np.ascontiguousarray(np.asarray(Wq)[:, sl]),
            "wk": np.ascontiguousarray(np.asarray(Wk)[:, sl]),
            "wv": np.ascontiguousarray(np.asarray(Wv)[:, sl]),
            "wrk": np.ascontiguousarray(np.asarray(Wrk)[:, sl]),
            "wrv": np.ascontiguousarray(np.asarray(Wrv)[:, sl]),
            "wp": np.ascontiguousarray(np.asarray(Wp)[sl, :]),
            "bq": np.ascontiguousarray(np.asarray(bq)[sl]),
            "bk": np.ascontiguousarray(np.asarray(bk)[sl]),
            "bv": np.ascontiguousarray(np.asarray(bv)[sl]),
            "brk": np.ascontiguousarray(np.asarray(brk)[sl]),
            "brv": np.ascontiguousarray(np.asarray(brv)[sl]),
        })
    return in_maps


def kernel(x, ref_feat, Wq, bq, Wk, bk, Wv, bv, Wrk, brk, Wrv, brv, Wp, bp):
    from concourse.bass_utils import run_bass_kernel_spmd

    nc = _get_program()
    in_maps = make_in_maps(x, ref_feat, Wq, bq, Wk, bk, Wv, bv,
                           Wrk, brk, Wrv, brv, Wp, bp)
    res = run_bass_kernel_spmd(nc, in_maps, list(range(NCORES))).results
    y = res[0]["out"].astype(np.float64)
    for c in range(1, NCORES):
        y += res[c]["out"]
    y += np.asarray(bp, dtype=np.float64)
    return y.astype(np.float32)



# revision 3
# speedup vs baseline: 1.4938x; 1.4938x over previous
"""Causal self-attention with reference-feature cross keys, on 8 TRN2 cores.

Sharding: tensor-parallel over heads. Core c owns global heads (2c, 2c+1),
i.e. columns [128c:128c+128) of Wq/Wk/Wv/Wrk/Wrv and rows [128c:128c+128)
of Wp. Each core returns a partial y (bf16); the host sums the 8 partials
and adds bp (the "all-reduce").

All-bf16 datapath (tol is 2e-2; bf16 end-to-end lands ~1e-3):
  - x/ref fed transposed [C, tokens] bf16 so contraction lands on SBUF
    partitions with plain DMAs; weights bf16. PSUM accumulation f32.
  - bk/brk dropped: score term q.bk is constant along keys -> softmax
    invariant (cancels in num/denom exactly).
  - Diagonal causal blocks are query-range restricted: for key block r of
    a 512-token chunk only queries t >= 128r participate; only the
    128x128 triangle straddling the diagonal needs an actual mask
    (multiplicative bf16 triangle on DVE). Fully-masked regions are never
    computed; PV accumulation covers them via the (full-range) ref block
    that starts the PSUM group.
  - exp without max-subtraction (|S/8| < ~3).

Per-core dataflow:
  qT/kT/vT [128, tok] = W.T @ xT            (accumulate over 8 C-chunks)
  v natural via PE transpose of vT blocks (+ ones column for the denom)
  S^T[s, t] = kT_blk.T @ qT_chunk           both heads -> one [128,2,512]
                                            PSUM pair (K=64 row packing)
  E = exp(S^T / 8)  (ACT, bf16 out)         triangle mask on DVE
  O^T (+denom row) += [V | 1].T @ E         accumulated over s-blocks
  O^T = po * (1/denom broadcast)            DVE recip + gpsimd broadcast
  y_part[t, :] = O^T_blk.T @ Wp_rows        emitted as PE "fillers"
                                            interleaved into the NEXT
                                            chunk's attention blocks so PE
                                            absorbs ACT latency gaps
"""
import sys

sys.path.insert(0, "/opt/trn_rl_repo")

import numpy as np

B, T, C = 4, 2048, 1024
TR = 512
D = 64
DC = 128          # per-core slice of C (2 heads x 64)
H_PER = 2
NCH = T // 512    # 512-token chunks per batch
NCORES = 8

_CACHE = {}


def _build_program(repeat=1, ablate="none"):
    import concourse.bacc as bacc
    import concourse.mybir as mybir
    import concourse.tile as tile
    from concourse.masks import make_identity

    F32 = mybir.dt.float32
    BF16 = mybir.dt.bfloat16
    AF = mybir.ActivationFunctionType
    OP = mybir.AluOpType

    nc = bacc.Bacc("TRN2", target_bir_lowering=False, debug=False,
                   num_devices=NCORES)

    xt_d = nc.dram_tensor("xt", [C, B * T], BF16, kind="ExternalInput").ap()
    rt_d = nc.dram_tensor("rt", [C, B * TR], BF16, kind="ExternalInput").ap()
    w_d = {}
    for nm in ("wq", "wk", "wv", "wrk", "wrv"):
        w_d[nm] = nc.dram_tensor(nm, [C, DC], BF16, kind="ExternalInput").ap()
    wp_d = nc.dram_tensor("wp", [DC, C], BF16, kind="ExternalInput").ap()
    b_d = {}
    for nm in ("bq", "bv", "brv"):
        b_d[nm] = nc.dram_tensor(nm, [DC], F32, kind="ExternalInput").ap()
    tri_d = nc.dram_tensor("tri", [128, 128], BF16, kind="ExternalInput").ap()
    out_d = nc.dram_tensor("out", [B, T, C], BF16, kind="ExternalOutput").ap()

    xt_v = xt_d.rearrange("(co p) t -> p co t", p=128)
    rt_v = rt_d.rearrange("(co p) t -> p co t", p=128)

    with tile.TileContext(nc) as tc:
        with (
            tc.tile_pool(name="const", bufs=1) as constp,
            tc.tile_pool(name="work", bufs=2) as work,
            tc.tile_pool(name="psum", bufs=1, space="PSUM") as psp,
        ):
            ident = constp.tile([128, 128], BF16)
            make_identity(nc, ident[:])
            tri = constp.tile([128, 128], BF16)
            nc.sync.dma_start(tri[:], tri_d)
            ones_col = constp.tile([128, 16], BF16)
            nc.any.memset(ones_col[:], 1.0)

            w_sb = {}
            for nm in ("wq", "wk", "wv", "wrk", "wrv"):
                w = constp.tile([128, 8, DC], BF16, name=f"{nm}_sb")
                nc.sync.dma_start(w[:], w_d[nm].rearrange("(co p) m -> p co m", p=128))
                w_sb[nm] = w
            wp_r = constp.tile([DC, C], BF16)
            nc.sync.dma_start(wp_r[:], wp_d)

            b_sb = {}
            for nm in ("bq", "bv", "brv"):
                bias = constp.tile([DC, 1], F32, name=f"{nm}_sb")
                nc.sync.dma_start(bias[:], b_d[nm].unsqueeze(1))
                b_sb[nm] = bias

            # PE filler queue: yproj work emitted lazily between attention
            # blocks of the following chunk to fill PE's ACT-wait gaps.
            fillers = []

            def emit_filler():
                if fillers:
                    fillers.pop(0)()

            def flush_fillers():
                while fillers:
                    fillers.pop(0)()

            import contextlib
            rep_ctx = tc.For_i(0, repeat, 1) if repeat > 1 else contextlib.nullcontext()
            with rep_ctx:
              for b in range(B):
                  qT = work.tile([128, NCH, 512], BF16, tag="qT")
                  kT = work.tile([128, NCH, 512], BF16, tag="kT")
                  v_sb = work.tile([128, 4 * NCH, 132], BF16, tag="vsb")
                  nc.vector.tensor_copy(v_sb[:, :, 64:65], ones_col[:, :, None])
                  nc.vector.tensor_copy(v_sb[:, :, 130:131], ones_col[:, :, None])
                  rkT = work.tile([128, 512], BF16, tag="rkT")
                  rv_sb = work.tile([128, 4, 132], BF16, tag="rvsb")
                  nc.vector.tensor_copy(rv_sb[:, :, 64:65], ones_col[:, 0:4, None])
                  nc.vector.tensor_copy(rv_sb[:, :, 130:131], ones_col[:, 0:4, None])
                  OT = work.tile([128, NCH, 512], BF16, tag="OT")

                  def project(xT, wname, bname, dst):
                      """dst[128, 512] (bf16) = W.T @ xT (+ bias)."""
                      if ablate == "noproj":
                          nc.vector.tensor_copy(dst, xT[:, 0, :])
                          return
                      pp = psp.tile([128, 512], F32, tag="pp", bufs=2)
                      for co in range(8):
                          nc.tensor.matmul(pp[:], w_sb[wname][:, co, :],
                                           xT[:, co, :], start=(co == 0), stop=(co == 7))
                      if bname is None:
                          nc.vector.tensor_copy(dst, pp[:])
                      else:
                          nc.vector.tensor_scalar_add(dst, pp[:], b_sb[bname][:])

                  def v_natural(vT, dst_vsb, j0):
                      """Transpose vT [128, 512] into v_sb blocks j0..j0+3 (+ones cols)."""
                      pt = psp.tile([128, 512], BF16, tag="pp", bufs=2)
                      for a in range(4):
                          nc.tensor.transpose(
                              pt[:, 128 * a:128 * (a + 1)],
                              vT[:, 128 * a:128 * (a + 1)], ident[:])
                      ptv = pt[:].rearrange("p (a m) -> p a m", a=4)
                      nc.vector.tensor_copy(dst_vsb[:, j0:j0 + 4, 0:64], ptv[:, :, 0:64])
                      nc.vector.tensor_copy(dst_vsb[:, j0:j0 + 4, 66:130], ptv[:, :, 64:128])

                  # ---- projections over x[b] ----
                  for n in range(NCH):
                      t0 = b * T + 512 * n
                      xT = work.tile([128, 8, 512], BF16, tag="xT", bufs=3)
                      nc.sync.dma_start(xT[:], xt_v[:, :, t0:t0 + 512])
                      project(xT, "wq", "bq", qT[:, n, :])
                      project(xT, "wk", None, kT[:, n, :])
                      vT = work.tile([128, 512], BF16, tag="vT")
                      project(xT, "wv", "bv", vT[:])
                      v_natural(vT, v_sb, 4 * n)
                  # ---- ref projections ----
                  refT = work.tile([128, 8, 512], BF16, tag="xT", bufs=3)
                  nc.sync.dma_start(refT[:], rt_v[:, :, b * TR:(b + 1) * TR])
                  project(refT, "wrk", None, rkT[:])
                  rvT = work.tile([128, 512], BF16, tag="vT")
                  project(refT, "wrv", "brv", rvT[:])
                  v_natural(rvT, rv_sb, 0)

                  # ---- attention: both heads per block (K=64 row packing) ----
                  if ablate == "noattn":
                      for c in range(NCH):
                          nc.vector.tensor_copy(OT[:, c, :], qT[:, c, :])
                      chunk_list = []
                  else:
                      chunk_list = list(range(NCH))
                  DEPTH = 2
                  for c in chunk_list:
                      po = psp.tile([128, 2, 512], F32, tag="po", bufs=1)
                      # (kind, j, qr): ref blocks (full), self full blocks,
                      # then diag blocks r=3..1 (query-restricted), diag r=0
                      # last (full range, carries the stop flag).
                      blocks = [("ref", jr, 0) for jr in range(4)]
                      blocks += [("self", j, 0) for j in range(4 * c)]
                      blocks += [("diag", 4 * c + r, 128 * r) for r in (3, 2, 1, 0)]
                      nb = len(blocks)
                      Es = {}

                      def s_stage(bi, c=c):
                          kind, j, qr = blocks[bi]
                          ps = psp.tile([128, 2, 512], F32, tag="s", bufs=2)
                          for h in (() if ablate == "nos" else range(H_PER)):
                              if kind == "ref":
                                  stat = rkT[64 * h:64 * (h + 1), 128 * j:128 * (j + 1)]
                              else:
                                  stat = kT[64 * h:64 * (h + 1), j // 4,
                                            128 * (j % 4):128 * (j % 4 + 1)]
                              nc.tensor.matmul(ps[:, h, qr:512], stat,
                                               qT[64 * h:64 * (h + 1), c, qr:512],
                                               start=True, stop=True)
                          E = work.tile([128, 2, 512], BF16, tag="E",
                                        bufs=DEPTH + 4)
                          if ablate == "noexp":
                              nc.vector.tensor_copy(E[:, :, qr:512], ps[:, :, qr:512])
                          else:
                              nc.scalar.activation(E[:, :, qr:512], ps[:, :, qr:512],
                                                   AF.Exp, scale=0.125)
                          if kind == "diag":
                              nc.vector.tensor_tensor(
                                  E[:, :, qr:qr + 128], E[:, :, qr:qr + 128],
                                  tri[:, None, :].to_broadcast((128, 2, 128)),
                                  OP.mult)
                          Es[bi] = E

                      def pv_stage(bi, c=c, po=po):
                          kind, j, qr = blocks[bi]
                          E = Es.pop(bi)
                          if ablate == "nopv":
                              if bi == 0:
                                  for h in range(H_PER):
                                      nc.tensor.matmul(po[0:65, h, :],
                                                       v_sb[:, 0, 66 * h:66 * h + 65],
                                                       E[:, h, :], start=True, stop=True)
                              return
                          for h in range(H_PER):
                              vstat = (rv_sb[:, j, 66 * h:66 * h + 65] if kind == "ref"
                                       else v_sb[:, j, 66 * h:66 * h + 65])
                              nc.tensor.matmul(po[0:65, h, qr:512], vstat,
                                               E[:, h, qr:512],
                                               start=(bi == 0), stop=(bi == nb - 1))

                      for bi in range(min(DEPTH, nb)):
                          s_stage(bi)
                      for bi in range(nb):
                          if bi + DEPTH < nb:
                              s_stage(bi + DEPTH)
                          pv_stage(bi)
                          emit_filler()
                      for h in range(H_PER):
                          rec = work.tile([1, 512], F32, tag="rec", bufs=2)
                          with nc.allow_low_precision(reason="softmax denom recip"):
                              nc.vector.reciprocal(rec[:], po[64:65, h, :])
                          bc_sb = work.tile([64, 512], F32, tag="bc", bufs=2)
                          nc.gpsimd.partition_broadcast(bc_sb[:], rec[:])
                          nc.vector.tensor_tensor(OT[64 * h:64 * (h + 1), c, :],
                                                  po[0:64, h, :], bc_sb[:], OP.mult)

                      # ---- queue output projection for this chunk ----
                      def make_filler(b=b, c=c, OT=OT):
                          def emit(a_half):
                              a, half = a_half
                              stat = OT[:, c, 128 * a:128 * (a + 1)]
                              py = psp.tile([128, 512], F32, tag="pp", bufs=2)
                              nc.tensor.matmul(py[:], stat,
                                               wp_r[:, 512 * half:512 * (half + 1)],
                                               start=True, stop=True)
                              y_sb = work.tile([128, 512], BF16, tag="y", bufs=3)
                              nc.vector.tensor_copy(y_sb[:], py[:])
                              t0 = 512 * c + 128 * a
                              nc.sync.dma_start(
                                  out_d[b, t0:t0 + 128, 512 * half:512 * (half + 1)],
                                  y_sb[:])
                          return [lambda ah=ah: emit(ah)
                                  for ah in ((a, hf) for a in range(4) for hf in range(2))]
                      fillers.extend(make_filler())
                  if ablate == "noattn":
                      for c in range(NCH):
                          def make_filler(b=b, c=c, OT=OT):
                              def emit(a_half):
                                  a, half = a_half
                                  stat = OT[:, c, 128 * a:128 * (a + 1)]
                                  py = psp.tile([128, 512], F32, tag="pp", bufs=2)
                                  nc.tensor.matmul(py[:], stat,
                                                   wp_r[:, 512 * half:512 * (half + 1)],
                                                   start=True, stop=True)
                                  y_sb = work.tile([128, 512], BF16, tag="y", bufs=3)
                                  nc.vector.tensor_copy(y_sb[:], py[:])
                                  t0 = 512 * c + 128 * a
                                  nc.sync.dma_start(
                                      out_d[b, t0:t0 + 128,
                                            512 * half:512 * (half + 1)],
                                      y_sb[:])
                              return [lambda ah=ah: emit(ah)
                                      for ah in ((a, hf) for a in range(4)
                                                 for hf in range(2))]
                          fillers.extend(make_filler())
              flush_fillers()

    nc.compile()
    return nc


def _get_program(repeat=1, ablate="none"):
    key = ("nc", repeat, ablate)
    if key not in _CACHE:
        _CACHE[key] = _build_program(repeat, ablate)
    return _CACHE[key]


def _make_tri():
    s = np.arange(128)[:, None]
    t = np.arange(128)[None, :]
    return (t >= s).astype(np.float32)


def make_in_maps(x, ref_feat, Wq, bq, Wk, bk, Wv, bv, Wrk, brk, Wrv, brv, Wp, bp):
    import ml_dtypes
    bf16 = ml_dtypes.bfloat16

    x = np.asarray(x, dtype=np.float32)
    ref_feat = np.asarray(ref_feat, dtype=np.float32)
    xt = np.ascontiguousarray(x.reshape(B * T, C).T).astype(bf16)
    rt = np.ascontiguousarray(ref_feat.reshape(B * TR, C).T).astype(bf16)
    tri = _make_tri().astype(bf16)

    in_maps = []
    for c in range(NCORES):
        sl = slice(DC * c, DC * (c + 1))
        in_maps.append({
            "xt": xt, "rt": rt, "tri": tri,
            "wq": np.ascontiguousarray(np.asarray(Wq)[:, sl]).astype(bf16),
            "wk": np.ascontiguousarray(np.asarray(Wk)[:, sl]).astype(bf16),
            "wv": np.ascontiguousarray(np.asarray(Wv)[:, sl]).astype(bf16),
            "wrk": np.ascontiguousarray(np.asarray(Wrk)[:, sl]).astype(bf16),
            "wrv": np.ascontiguousarray(np.asarray(Wrv)[:, sl]).astype(bf16),
            "wp": np.ascontiguousarray(np.asarray(Wp)[sl, :]).astype(bf16),
            "bq": np.ascontiguousarray(np.asarray(bq)[sl]).astype(np.float32),
            "bv": np.ascontiguousarray(np.asarray(bv)[sl]).astype(np.float32),
            "brv": np.ascontiguousarray(np.asarray(brv)[sl]).astype(np.float32),
        })
    return in_maps


def kernel(x, ref_feat, Wq, bq, Wk, bk, Wv, bv, Wrk, brk, Wrv, brv, Wp, bp):
    from concourse.bass_utils import run_bass_kernel_spmd

    nc = _get_program()
    in_maps = make_in_maps(x, ref_feat, Wq, bq, Wk, bk, Wv, bv,
                           Wrk, brk, Wrv, brv, Wp, bp)
    res = run_bass_kernel_spmd(nc, in_maps, list(range(NCORES))).results
    y = res[0]["out"].astype(np.float64)
    for c in range(1, NCORES):
        y += res[c]["out"].astype(np.float64)
    y += np.asarray(bp, dtype=np.float64)
    return y.astype(np.float32)
